# revision 1
# baseline (speedup 1.0000x reference)
"""BitLinear forward (ternary groupwise-quantized linear) on 8 Trainium2 NeuronCores.

Computation:  out = x @ ternary_quantize_groupwise(weight).T
  x: [2, 2048, 4096] f32, weight: [4096, 4096] f32, group=128 along in_features.

Sharding (tensor-parallel, per hint): weight rows (out_features) are split
across 8 cores (512 rows each); x is replicated; each core computes its
[4096, 512] output slice; host concatenates along the feature dim.

Device kernel per core:
  - quantize w shard on-chip: per-group absmean scale (f32, matching the
    reference's thresholding exactly up to reduction order), ternary values
    materialized as q * scale rounded to fp16.
  - x is shipped as an fp16 hi/lo pair (x == hi + lo + O(2^-22)); both halves
    are multiplied by the same fp16 quantized weight on the PE array and
    accumulated in the same fp32 PSUM bank, giving near-fp32 accuracy at
    16-bit matmul throughput.
  - x tiles and the quantized weight are transposed on-chip with the DMA
    xbar transpose (contraction dim must sit on SBUF partitions for the PE).
"""

import os
from contextlib import ExitStack

import numpy as np

import concourse.bass as bass
import concourse.bacc as bacc
import concourse.mybir as mybir
import concourse.tile as tile

# Problem shapes (hardcoded per contract; kernel.py must be self-contained).
B, S, DIM_D, DIM_O = 2, 2048, 4096, 4096
T = B * S                 # 4096 tokens
NCORES = 8
O_SHARD = DIM_O // NCORES  # 512 out features per core
P = 128                    # SBUF partitions / PE array dim
GROUP = 128                # quant group size along in_features
EPS = 1e-8
THRESHOLD = 0.5

f32 = mybir.dt.float32
f16 = mybir.dt.float16
bf16 = mybir.dt.bfloat16


DEFAULT_CFG = dict(
    # x_hi in bf16: its f16 residual straddles the f16 subnormal range; the
    # bf16 residual (~2^-9 |x|) stays comfortably normal in f16.
    xh_dtype="bfloat16",
    evac="scalar",        # ACT sits closer to PSUM; frees DVE
    store_ring="scalar",  # plain DMAs coexist fine with transposes elsewhere
    w_load="gpsimd",      # own SWDGE queue: never queues behind transposes
    x_load="scalar",      # ACT ring is idle during the ramp
    wnat_bufs=2,
    abs_bufs=2,           # double-buffer |w| so ACT(ot+1) overlaps DVE(ot)
    q_chunks=2,           # quantize in D/2 chunks for finer pipeline overlap
    psum_bufs=6,
    xT_bufs=2,            # buffers of [P, G, T_SPAN] per half
    t_span=256,           # tokens per x load slice
    # Ship x pre-transposed ([D, T]) from the host: on-device xbar transposes
    # cost ~10us/2MB of serialized sequencer ucode descriptor generation and
    # corrupt data when issued from both HWDGE rings; a plain strided load of
    # the pre-transposed layout runs at DMA line rate with none of that.
    host_transpose=True,
)


def _emit(ctx, tc, xh, xl, w, out, T_, D_, O_, cfg):
    """Emit the per-core program. xh/xl: [T_, D_] f16 DRAM; w: [O_, D_] f32;
    out: [T_, O_] f32."""
    nc = tc.nc
    xl_eng = getattr(nc, cfg.get("xl_ring", "sync"))
    store_eng = getattr(nc, cfg["store_ring"])
    G = D_ // P            # number of d-chunks == quant groups along D
    OT = O_ // P           # o-tiles of the weight shard
    TT = T_ // P           # token tiles
    NBLK = min(O_, 512)    # psum free dim (one bank at 512 f32)
    NB = O_ // NBLK
    dual = xl is not None

    wpool = ctx.enter_context(tc.tile_pool(name="wnat", bufs=cfg["wnat_bufs"]))
    qpool = ctx.enter_context(tc.tile_pool(name="quant", bufs=2))
    apool = ctx.enter_context(tc.tile_pool(name="absw", bufs=cfg["abs_bufs"]))
    spool = ctx.enter_context(tc.tile_pool(name="stats", bufs=2))
    wqT_pool = ctx.enter_context(tc.tile_pool(name="wqT", bufs=1))
    xT_pool = ctx.enter_context(tc.tile_pool(name="xT", bufs=cfg["xT_bufs"]))
    opool = ctx.enter_context(tc.tile_pool(name="osb", bufs=4))
    psum = ctx.enter_context(
        tc.tile_pool(name="psum", bufs=cfg["psum_bufs"], space="PSUM"))

    # ---- Phase 1: quantize weight shard, produce wqT [d: P x G, o: O_] f16
    # Processed in QCH chunks along D for a fine-grained ACT/DVE pipeline —
    # this chain gates the first matmul, so its latency is the startup ramp.
    QCH = cfg["q_chunks"]
    DC = D_ // QCH
    GC = G // QCH
    wqT = wqT_pool.tile([P, G, O_], f16, tag="wqT")
    for ot in range(OT):
        for h in range(QCH):
            dsl = slice(h * DC, (h + 1) * DC)
            wt = wpool.tile([P, DC], f32, tag="wnat")
            getattr(nc, cfg["w_load"]).dma_start(
                wt[:], w[ot * P:(ot + 1) * P, dsl])

            # ACT (off DVE critical path): |w|, sign(w)
            abs_w = apool.tile([P, DC], f32, tag="abs_w")
            nc.scalar.activation(abs_w[:], wt[:], mybir.ActivationFunctionType.Abs)
            sgn = qpool.tile([P, DC], f16, tag="sgn")
            nc.scalar.activation(sgn[:], wt[:], mybir.ActivationFunctionType.Sign)

            red = spool.tile([P, GC], f32, tag="red")
            nc.vector.tensor_reduce(
                red[:], abs_w[:].rearrange("p (g j) -> p g j", j=GROUP),
                axis=mybir.AxisListType.X, op=mybir.AluOpType.add,
            )
            # thr = 0.5*max(red/128, EPS) = max(red/256, EPS/2) (exact in f32)
            thr = spool.tile([P, GC], f32, tag="thr")
            nc.vector.tensor_scalar(
                thr[:], red[:], 1.0 / 256.0, EPS / 2.0,
                op0=mybir.AluOpType.mult, op1=mybir.AluOpType.max,
            )
            # scale rounded to f16 (the only precision loss on the weight side)
            s16 = spool.tile([P, GC], f16, tag="s16")
            nc.vector.tensor_scalar(
                s16[:], red[:], 1.0 / 128.0, EPS,
                op0=mybir.AluOpType.mult, op1=mybir.AluOpType.max,
            )
            # ACT: per-group scale broadcast
            s16row = qpool.tile([P, DC], f16, tag="s16row")
            nc.scalar.activation(
                s16row[:].rearrange("p (g j) -> p g j", j=GROUP),
                s16[:].unsqueeze(2).broadcast_to((P, GC, GROUP)),
                mybir.ActivationFunctionType.Copy,
            )
            # DVE: c = (|w| > thr); q = c * sign(w); wq = q * scale16
            c = qpool.tile([P, DC], f16, tag="c")
            nc.vector.tensor_tensor(
                c[:].rearrange("p (g j) -> p g j", j=GROUP),
                abs_w[:].rearrange("p (g j) -> p g j", j=GROUP),
                thr[:].unsqueeze(2).broadcast_to((P, GC, GROUP)),
                op=mybir.AluOpType.is_gt,
            )
            q = qpool.tile([P, DC], f16, tag="q")
            nc.vector.tensor_tensor(q[:], c[:], sgn[:], op=mybir.AluOpType.mult)
            wqn = qpool.tile([P, DC], f16, tag="wqn")
            nc.vector.tensor_tensor(wqn[:], q[:], s16row[:], op=mybir.AluOpType.mult)
            # wqT[p, h*GC+a, ot*P+b] = wqn[b, a*P+p]  (xbar sb2sb transpose).
            # On the sync ring: ALL DMA-transposes share one ring — transposes
            # issued concurrently from both HWDGE rings corrupt data on HW.
            nc.sync.dma_start_transpose(
                wqT[:, h * GC:(h + 1) * GC, ot * P:(ot + 1) * P], wqn[:])

    # ---- Phase 2: stream token spans: load xT slices, matmul, store
    TSPAN = min(cfg["t_span"], T_)
    SPANS = T_ // TSPAN
    PER = TSPAN // P
    xh_dt = getattr(mybir.dt, cfg["xh_dtype"])
    for s in range(SPANS):
        tspan_sl = slice(s * TSPAN, (s + 1) * TSPAN)
        x_eng = getattr(nc, cfg["x_load"])
        xTh = xT_pool.tile([P, G, TSPAN], xh_dt, tag="xTh")
        if cfg["host_transpose"]:
            # xh is [D, T] on the host side; strided line-rate load
            x_eng.dma_start(
                xTh[:], xh[:, tspan_sl].rearrange("(g p) t -> p g t", p=P))
        else:
            nc.sync.dma_start_transpose(xTh[:], xh[s * TSPAN:(s + 1) * TSPAN, :])
        if dual:
            xTl = xT_pool.tile([P, G, TSPAN], f16, tag="xTl")
            if cfg["host_transpose"]:
                x_eng.dma_start(
                    xTl[:], xl[:, tspan_sl].rearrange("(g p) t -> p g t", p=P))
            else:
                xl_eng.dma_start_transpose(xTl[:], xl[s * TSPAN:(s + 1) * TSPAN, :])
        for sub in range(PER):
            tt = s * PER + sub
            tsl = slice(sub * P, (sub + 1) * P)
            for nb in range(NB):
                osl = slice(nb * NBLK, (nb + 1) * NBLK)
                ps = psum.tile([P, NBLK], f32, tag="ps")
                for g in range(G):
                    nc.tensor.matmul(
                        ps[:], lhsT=xTh[:, g, tsl], rhs=wqT[:, g, osl],
                        start=(g == 0), stop=(g == G - 1 and not dual),
                    )
                    if dual:
                        nc.tensor.matmul(
                            ps[:], lhsT=xTl[:, g, tsl], rhs=wqT[:, g, osl],
                            start=False, stop=(g == G - 1),
                        )
                osb = opool.tile([P, NBLK], f32, tag="osb")
                if cfg["evac"] == "vector":
                    nc.vector.tensor_copy(osb[:], ps[:])
                else:
                    nc.scalar.copy(osb[:], ps[:])
                store_eng.dma_start(out[tt * P:(tt + 1) * P, osl], osb[:])


def build_nc(T_=T, D_=DIM_D, O_=O_SHARD, dual=True, cfg=None):
    cfg = {**DEFAULT_CFG, **(cfg or {})}
    # Bacc (not raw Bass): its compile() legalizes sync waits (walrus allows
    # at most 1 wait per DMA instruction) and fuses nops.
    nc = bacc.Bacc("TRN2", target_bir_lowering=False, debug=False)
    xh_dt = getattr(mybir.dt, cfg["xh_dtype"])
    xshape = [D_, T_] if cfg["host_transpose"] else [T_, D_]
    xh = nc.declare_dram_parameter("xh", xshape, xh_dt, isOutput=False)
    xl = nc.declare_dram_parameter("xl", xshape, f16, isOutput=False) if dual else None
    w = nc.declare_dram_parameter("w", [O_, D_], f32, isOutput=False)
    out = nc.declare_dram_parameter("out", [T_, O_], f32, isOutput=True)
    with tile.TileContext(nc) as tc:
        with ExitStack() as ctx:
            _emit(ctx, tc, xh.ap(), xl.ap() if dual else None, w.ap(), out.ap(),
                  T_, D_, O_, cfg)
    nc.compile()
    return nc


def prepare_inputs(x, weight, dual=True, cfg=None):
    import ml_dtypes

    cfg = {**DEFAULT_CFG, **(cfg or {})}
    xh_np = (ml_dtypes.bfloat16 if cfg["xh_dtype"] == "bfloat16" else np.float16)
    xf = np.ascontiguousarray(np.asarray(x, dtype=np.float32).reshape(T, DIM_D))
    wf = np.ascontiguousarray(np.asarray(weight, dtype=np.float32))
    xh = xf.astype(xh_np)
    xlo = (xf - xh.astype(np.float32)).astype(np.float16) if dual else None
    if cfg["host_transpose"]:
        xh = np.ascontiguousarray(xh.T)
        if dual:
            xlo = np.ascontiguousarray(xlo.T)
    in_maps = []
    for c in range(NCORES):
        m = {
            "xh": xh,
            "w": np.ascontiguousarray(wf[c * O_SHARD:(c + 1) * O_SHARD]),
        }
        if dual:
            m["xl"] = xlo
        in_maps.append(m)
    return in_maps


def run(x, weight, dual=True, trace=False, cfg=None, **kwargs):
    from concourse.bass_utils import run_bass_kernel_spmd

    if not dual:
        # single-pass: f16 x beats bf16 3x on accuracy at the same speed;
        # 512-token spans halve DMA count (1KB partition lines)
        cfg = {"xh_dtype": "float16", "t_span": 512, **(cfg or {})}
    nc = build_nc(dual=dual, cfg=cfg)
    in_maps = prepare_inputs(x, weight, dual=dual, cfg=cfg)
    res = run_bass_kernel_spmd(
        nc, in_maps, core_ids=list(range(NCORES)), trace=trace, **kwargs
    )
    outs = [np.asarray(res.results[c]["out"]) for c in range(NCORES)]
    full = np.concatenate(outs, axis=1).reshape(B, S, DIM_O)
    return full, res


def kernel(x, weight):
    full, _ = run(x, weight, dual=True, trace=False)
    return full.astype(np.float32)



# revision 15
# speedup vs baseline: 1.1259x; 1.1259x over previous
"""BitLinear forward (ternary groupwise-quantized linear) on 8 Trainium2 NeuronCores.

Computation:  out = x @ ternary_quantize_groupwise(weight).T
  x: [2, 2048, 4096] f32, weight: [4096, 4096] f32, group=128 along in_features.

Sharding (tensor-parallel): weight rows (out_features) split across 8 cores
(512 rows each); x replicated; each core computes its [4096, 512] output
slice; host concatenates along the feature dim.

v4 kernel (single-pass f16, quant/matmul/DMA overlap):
  - x ships f16, host-pre-transposed to [D, T] (rel err ~3e-4 vs the 2e-2
    gate; dual hi/lo passes unnecessary).
  - w shard quantized on-chip in [o-part, d-free] layout (per-group f32
    absmean scale, exact threshold compare), f16-cast, DMA-xbar transposed
    into wqT [d-part, o-free].
  - flipped matmul roles: the quantized weight tile [128d, 128o] is the PE
    stationary operand, x [128d, 512t] streams; each matmul needs only ONE
    o-tile of wqT, so the PE starts after ~1/8 of the quantization. PSUM
    holds [o, t]; output stores transposed [O_shard, T] f16, fixed on host.
  - tokens processed in 1024-token spans, double-buffered: big enough that
    the o-tile pass cadence (~14 us) roughly matches the w-load-limited
    quant cadence, small enough that the first pass only needs 8.4 MB of x.
  - each DMA queue sustains only ~140 GB/s, so bulk traffic is placed per
    queue: w (8.4 MB) exclusively on gpsimd; x even-g slices on scalar;
    span-0 odd-g slices on sync woven around the wq transposes (with
    explicit ordering deps so the scheduler cannot hoist bulk x above the
    transposes that gate the PE); span 1+ odd-g slices on gpsimd (idle
    after w); output stores on sync after all transposes.
"""

import os
from contextlib import ExitStack

import numpy as np

import concourse.bass as bass
import concourse.bacc as bacc
import concourse.mybir as mybir
import concourse.tile as tile
from concourse.tile import add_dep_helper

# Problem shapes (hardcoded per contract; kernel.py must be self-contained).
B, S, DIM_D, DIM_O = 2, 2048, 4096, 4096
T = B * S                 # 4096 tokens
NCORES = 8
O_SHARD = DIM_O // NCORES  # 512 out features per core
P = 128                    # SBUF partitions / PE array dim
GROUP = 128                # quant group size along in_features
EPS = 1e-8

f32 = mybir.dt.float32
f16 = mybir.dt.float16

DEFAULT_CFG = dict(
    q_chunks=4,        # quant compute chunks per o-tile (pipeline grain)
    w_chunks=2,        # w DMA chunks per o-tile (8KB rows/packets)
    t_span=1024,       # token span size (xq double-buffered)
    nblk=512,          # moving/psum free size (one PSUM bank of f32)
    psum_bufs=8,
    osb_bufs=6,
    out_dtype="float16",
    x0_scalar_upfront=6,
    x0_sync_upfront=8,
)


def _emit(ctx, tc, xh, w, out, T_, D_, O_, cfg):
    """Per-core program. xh: [D_, T_] f16 DRAM; w: [O_, D_] f32 DRAM;
    out: [O_, T_] f16 DRAM (transposed output)."""
    nc = tc.nc
    G = D_ // P                # 32 groups along D
    OT = O_ // P               # 4 o-tiles
    QCH = cfg["q_chunks"]
    DC = D_ // QCH             # quant compute chunk width
    GC = G // QCH              # groups per compute chunk
    TS = cfg["t_span"]
    NS = T_ // TS              # spans
    NBLK = cfg["nblk"]
    TB = TS // NBLK            # token blocks (psum banks) per pass
    out_dt = getattr(mybir.dt, cfg["out_dtype"])
    WCH = cfg["w_chunks"]
    WDC = D_ // WCH
    HPW = QCH // WCH           # compute chunks per w DMA chunk

    sb = ctx.enter_context(tc.tile_pool(name="sb", bufs=1))
    psum = ctx.enter_context(
        tc.tile_pool(name="psum", bufs=cfg["psum_bufs"], space="PSUM"))

    wqT = sb.tile([P, G, O_], f16, tag="wqT", bufs=1)
    # span x buffers cycle through 2 slots via the pool
    xq = [sb.tile([P, G, TS], f16, tag="xq", bufs=2, name=f"xq{s}")
          for s in range(NS)]

    def x_slice(eng, s, g):
        return eng.dma_start(
            xq[s][:, g, :], xh[g * P:(g + 1) * P, s * TS:(s + 1) * TS])

    # ---- Phase Q: quantize w shard o-tile by o-tile, producing wqT.
    x0_scalar = iter(range(0, G, 2))
    x0_sync = iter(range(1, G, 2))

    def issue_x0_scalar(n):
        for _ in range(n):
            g = next(x0_scalar, None)
            if g is not None:
                x_slice(nc.scalar, 0, g)

    issue_x0_scalar(cfg["x0_scalar_upfront"])
    for _ in range(cfg["x0_sync_upfront"]):
        g = next(x0_sync, None)
        if g is not None:
            x_slice(nc.sync, 0, g)

    # all w DMAs upfront on gpsimd; wt double-buffer paces them naturally
    wt_tiles = {}
    for ot in range(OT):
        for wh in range(WCH):
            wt = sb.tile([P, WDC], f32, tag="wt", bufs=2, name=f"wt{ot}_{wh}")
            nc.gpsimd.dma_start(
                wt[:], w[ot * P:(ot + 1) * P, wh * WDC:(wh + 1) * WDC])
            wt_tiles[(ot, wh)] = wt

    last_transpose = None
    for ot in range(OT):
        for h in range(QCH):
            wt = wt_tiles[(ot, h // HPW)]
            wtv = wt[:, (h % HPW) * DC:(h % HPW + 1) * DC]

            abs_w = sb.tile([P, DC], f32, tag="abs_w", bufs=2)
            nc.scalar.activation(abs_w[:], wtv, mybir.ActivationFunctionType.Abs)
            sgn = sb.tile([P, DC], f16, tag="sgn", bufs=2)
            nc.scalar.activation(sgn[:], wtv, mybir.ActivationFunctionType.Sign)
            issue_x0_scalar(2)

            red = sb.tile([P, GC], f32, tag="red", bufs=2)
            nc.vector.tensor_reduce(
                red[:], abs_w[:].rearrange("p (g j) -> p g j", j=GROUP),
                axis=mybir.AxisListType.X, op=mybir.AluOpType.add,
            )
            # thr = 0.5*max(red/128, EPS) = max(red/256, EPS/2) (exact in f32)
            thr = sb.tile([P, GC], f32, tag="thr", bufs=2)
            nc.vector.tensor_scalar(
                thr[:], red[:], 1.0 / 256.0, EPS / 2.0,
                op0=mybir.AluOpType.mult, op1=mybir.AluOpType.max,
            )
            # scale rounded to f16 (the only weight-side precision loss)
            s16 = sb.tile([P, GC], f16, tag="s16", bufs=2)
            nc.vector.tensor_scalar(
                s16[:], red[:], 1.0 / 128.0, EPS,
                op0=mybir.AluOpType.mult, op1=mybir.AluOpType.max,
            )
            # c = (|w| > thr); q = c*sign(w); wq = q*scale16 (broadcasts via
            # stride-0 APs; no materialized scale rows)
            c = sb.tile([P, DC], f16, tag="c", bufs=2)
            nc.vector.tensor_tensor(
                c[:].rearrange("p (g j) -> p g j", j=GROUP),
                abs_w[:].rearrange("p (g j) -> p g j", j=GROUP),
                thr[:].unsqueeze(2).broadcast_to((P, GC, GROUP)),
                op=mybir.AluOpType.is_gt,
            )
            q = sb.tile([P, DC], f16, tag="q", bufs=2)
            nc.vector.tensor_tensor(q[:], c[:], sgn[:], op=mybir.AluOpType.mult)
            wq = sb.tile([P, DC], f16, tag="wq", bufs=2)
            nc.vector.tensor_tensor(
                wq[:].rearrange("p (g j) -> p g j", j=GROUP),
                q[:].rearrange("p (g j) -> p g j", j=GROUP),
                s16[:].unsqueeze(2).broadcast_to((P, GC, GROUP)),
                op=mybir.AluOpType.mult,
            )
            # wqT[p, h*GC+a, ot*P+b] = wq[b, a*P+p]; all transposes stay on
            # the sync ring (two-ring transposes corrupt data on HW).
            t_inst = nc.sync.dma_start_transpose(
                wqT[:, h * GC:(h + 1) * GC, ot * P:(ot + 1) * P], wq[:])
            last_transpose = t_inst
            # weave one span-0 odd-g slice behind each transpose; the
            # explicit edge stops the scheduler from hoisting bulk x above
            # the transpose that gates the PE.
            g_odd = next(x0_sync, None)
            if g_odd is not None:
                x_inst = x_slice(nc.sync, 0, g_odd)
                add_dep_helper(x_inst.ins, t_inst.ins, sync=False,
                               reason="keep transposes ahead of bulk x")
    issue_x0_scalar(G)
    for g_odd in x0_sync:
        x_inst = x_slice(nc.sync, 0, g_odd)
        add_dep_helper(x_inst.ins, last_transpose.ins, sync=False,
                       reason="keep transposes ahead of bulk x")

    # ---- Phase M: matmul passes. Span s+1 x loads are emitted after span
    # s's matmuls (dep tracking orders only against already-emitted
    # readers); evens on scalar, odds on gpsimd (idle once w is loaded).
    # WAR on the 2-deep xq buffer ring is tracked by the pool.
    for s in range(NS):
        tbase = s * TS
        if s + 1 < NS:
            nxt = s + 1
            for g in range(G):
                eng = nc.scalar if g % 2 == 0 else nc.gpsimd
                x_inst = x_slice(eng, nxt, g)
                if eng is nc.sync:
                    add_dep_helper(x_inst.ins, last_transpose.ins, sync=False,
                                   reason="x after transposes")
        for ot2 in range(OT):
            osl = slice(ot2 * P, (ot2 + 1) * P)
            ps = [psum.tile([P, NBLK], f32, tag="ps", name=f"ps{s}_{ot2}_{tb}")
                  for tb in range(TB)]
            for g in range(G):
                for tb in range(TB):
                    nc.tensor.matmul(
                        ps[tb][:], lhsT=wqT[:, g, osl],
                        rhs=xq[s][:, g, tb * NBLK:(tb + 1) * NBLK],
                        start=(g == 0), stop=(g == G - 1),
                    )
            for tb in range(TB):
                osb = sb.tile([P, NBLK], out_dt, tag="osb", bufs=cfg["osb_bufs"])
                nc.scalar.copy(osb[:], ps[tb][:])
                st = nc.sync.dma_start(
                    out[osl, tbase + tb * NBLK:tbase + (tb + 1) * NBLK], osb[:])
                add_dep_helper(st.ins, last_transpose.ins, sync=False,
                               reason="stores after transposes")


def build_nc(T_=T, D_=DIM_D, O_=O_SHARD, cfg=None):
    cfg = {**DEFAULT_CFG, **(cfg or {})}
    nc = bacc.Bacc("TRN2", target_bir_lowering=False, debug=False)
    xh = nc.declare_dram_parameter("xh", [D_, T_], f16, isOutput=False)
    w = nc.declare_dram_parameter("w", [O_, D_], f32, isOutput=False)
    out_dt = getattr(mybir.dt, cfg["out_dtype"])
    out = nc.declare_dram_parameter("out", [O_, T_], out_dt, isOutput=True)
    with tile.TileContext(nc) as tc:
        with ExitStack() as ctx:
            _emit(ctx, tc, xh.ap(), w.ap(), out.ap(), T_, D_, O_, cfg)
    nc.compile()
    return nc


def prepare_inputs(x, weight):
    xf = np.ascontiguousarray(np.asarray(x, dtype=np.float32).reshape(T, DIM_D))
    wf = np.ascontiguousarray(np.asarray(weight, dtype=np.float32))
    xh = np.ascontiguousarray(xf.astype(np.float16).T)
    in_maps = []
    for c in range(NCORES):
        in_maps.append({
            "xh": xh,
            "w": np.ascontiguousarray(wf[c * O_SHARD:(c + 1) * O_SHARD]),
        })
    return in_maps


def run(x, weight, trace=False, cfg=None, **kwargs):
    from concourse.bass_utils import run_bass_kernel_spmd

    nc = build_nc(cfg=cfg)
    in_maps = prepare_inputs(x, weight)
    res = run_bass_kernel_spmd(
        nc, in_maps, core_ids=list(range(NCORES)), trace=trace, **kwargs
    )
    # out is [O_shard, T] per core; transpose + concat along features
    outs = [np.asarray(res.results[c]["out"]).astype(np.float32).T
            for c in range(NCORES)]
    full = np.concatenate(outs, axis=1).reshape(B, S, DIM_O)
    return full, res


def kernel(x, weight):
    full, _ = run(x, weight, trace=False)
    return full.astype(np.float32)
